# revision 45
# baseline (speedup 1.0000x reference)
"""Self-contained Trainium2 Bass kernel for nn_Attention_395136991961.

Dense multi-head attention (B=8, N=1024, C=1024, H=16, D=64) with RoPE,
full materialized softmax, and output projection.

Sharding: data-parallel over batch B across the 8 NeuronCores (one batch
element per core, weights replicated, no collectives).

v2 design (vs the f32r baseline at ~467us):
  - everything bf16 into the PE (measured numerics: ~4e-3 relmax, gate 2e-2)
  - q/k produced TRANSPOSED directly by the QKV matmul (w stationary,
    x^T moving) -> zero PE transposes, no PE-waits-DVE serialization
  - RoPE applied in the [d-on-partitions, n] layout: the rotate-half
    partner lives 16 partitions away inside a 32-group thanks to a host-
    side permutation of the q/k weight columns (contraction order of
    q.k is permutation invariant), so one DVE stream_shuffle + two
    multiplies + one add do RoPE at full partition utilization
  - softmax exp in [128, 2x512] ops (2 PSUM banks) straight PSUM->SBUF
  - attention blocks software-pipelined with the QKV chains of later
    head-pairs so the exp-gated PE gaps are filled with matmul work
  - PSUM budget: 2 banks chains (qkv+proj shared), 4 banks logit groups,
    2 banks PV accumulators = 8
"""

import sys

if "/opt/trn_rl_repo" not in sys.path:
    sys.path.insert(0, "/opt/trn_rl_repo")

import numpy as np

import concourse.tile as tile
import concourse.mybir as mybir
from concourse import bacc
from concourse.bass_utils import run_bass_kernel_spmd

F32 = mybir.dt.float32
BF16 = mybir.dt.bfloat16
AF = mybir.ActivationFunctionType
OP = mybir.AluOpType

N_CORES = 8
C = 1024
H = 16
D = 64
NCT = C // 128          # contraction chunks (8)
SCALE = float(D) ** -0.5

# rotate-half partner permutation: partition p (within a 64-half) holds
# d = PERM64[p]; partner (d <-> d+-32) sits at p XOR 16 (same 32-group)
PERM64 = (
    list(range(0, 16)) + list(range(32, 48))
    + list(range(16, 32)) + list(range(48, 64))
)
SHUF_MASK = [i ^ 16 for i in range(32)]

PROFILE = False
TRACE_DIR = None
DEBUG = False
LAST_EXEC_NS = None
_CACHE = {}


def build(n_tok):
    ntile = n_tok // 128
    nmc = n_tok // 512           # token chunks for logits moving dim

    nc = bacc.Bacc("TRN2", target_bir_lowering=False, debug=False, num_devices=1)

    # partition-major layouts (host pre-permuted) for large-descriptor DMAs
    xP = nc.dram_tensor("xP", [128, NCT * n_tok], BF16, kind="ExternalInput").ap()
    wv = nc.dram_tensor("wv", [128, NCT * C], BF16, kind="ExternalInput").ap()
    wqk = nc.dram_tensor(
        "wqk", [128, NCT * 2 * C], BF16, kind="ExternalInput"
    ).ap()
    pwP = nc.dram_tensor("pwP", [128, NCT * C], BF16, kind="ExternalInput").ap()
    pbias = nc.dram_tensor("pbias", [1, C], BF16, kind="ExternalInput").ap()
    cosT = nc.dram_tensor("cosT", [128, n_tok], BF16, kind="ExternalInput").ap()
    sinT = nc.dram_tensor("sinT", [128, n_tok], BF16, kind="ExternalInput").ap()
    y = nc.dram_tensor("y", [n_tok, C], F32, kind="ExternalOutput").ap()
    if DEBUG:
        dbg_qkT = nc.dram_tensor(
            "dbg_qkT", [128, 16 * n_tok], BF16, kind="ExternalOutput"
        ).ap()
        dbg_v = nc.dram_tensor(
            "dbg_v", [128, (n_tok // 128) * H * (D + 1)], BF16,
            kind="ExternalOutput",
        ).ap()
        dbg_oT = nc.dram_tensor(
            "dbg_oT", [128, NCT * n_tok], BF16, kind="ExternalOutput"
        ).ap()
        dbg_pT = nc.dram_tensor(
            "dbg_pT", [128, (n_tok // 128) * 2 * 512], BF16,
            kind="ExternalOutput",
        ).ap()

    xP_t = xP.rearrange("p (t n) -> p t n", t=NCT)
    wv_t = wv.rearrange("p (t j) -> p t j", t=NCT)
    wqk_t = wqk.rearrange("p (t j) -> p t j", t=NCT)
    pwP_t = pwP.rearrange("p (t a e) -> p t a e", t=NCT, a=2)

    with tile.TileContext(nc) as tc:
        with (
            tc.tile_pool(name="persist", bufs=1) as pp,
            tc.tile_pool(name="ptp", bufs=2) as ptp,
            tc.tile_pool(name="qsp", bufs=2) as qsp,
            tc.tile_pool(name="rtp", bufs=2) as rtp,
            tc.tile_pool(name="nrm", bufs=2) as nrm,
            tc.tile_pool(name="ypool", bufs=2) as yp,
            tc.tile_pool(name="ps1", bufs=2, space="PSUM") as ps1,
            tc.tile_pool(name="grp", bufs=2, space="PSUM") as grp,
            tc.tile_pool(name="pop", bufs=2, space="PSUM") as pop,
        ):
            # ---------------- persistent tiles + loads ----------------
            x_sb = pp.tile([128, NCT, n_tok], BF16, tag="x")
            w_sb = pp.tile([128, NCT, 3 * C], BF16, tag="w")
            qkT = pp.tile([128, 16, n_tok], BF16, tag="qkT")
            v_sb = pp.tile([128, ntile, H, D + 1], BF16, tag="v")
            oT = pp.tile([128, NCT, n_tok], BF16, tag="oT")
            pw_sb = pp.tile([128, NCT, 2, 512], BF16, tag="pw")
            cos_sb = pp.tile([128, n_tok], BF16, tag="cos")
            sin_sb = pp.tile([128, n_tok], BF16, tag="sin")
            bias_b = pp.tile([128, C], BF16, tag="biasb")

            nc.scalar.dma_start(cos_sb[:], cosT[:])
            nc.scalar.dma_start(sin_sb[:], sinT[:])
            nc.scalar.dma_start(bias_b[0:1, :], pbias[:])
            nc.gpsimd.partition_broadcast(bias_b[:, :], bias_b[0:1, :])
            nc.scalar.dma_start(x_sb[:], xP_t)
            # v columns first, then q/k in consumption order
            nc.sync.dma_start(w_sb[:, :, 2 * C : 3 * C], wv_t)
            nc.sync.dma_start(w_sb[:, :, 0:C], wqk_t[:, :, 0:C])
            nc.sync.dma_start(w_sb[:, :, C : 2 * C], wqk_t[:, :, C : 2 * C])
            nc.scalar.dma_start(pw_sb[:], pwP_t)
            nc.vector.memset(v_sb[:, :, :, D : D + 1], 1.0)

            # ---------------- v chains (stationary x, moving w) ----------
            for t in range(ntile):
                for half in range(2):
                    ps = ps1.tile([128, 512], F32, tag="ps1")
                    j0 = 2 * C + half * 512
                    for ct in range(NCT):
                        nc.tensor.matmul(
                            ps[:],
                            x_sb[:, ct, t * 128 : (t + 1) * 128],
                            w_sb[:, ct, j0 : j0 + 512],
                            start=(ct == 0),
                            stop=(ct == NCT - 1),
                        )
                    nc.scalar.copy(
                        v_sb[:, t, half * 8 : half * 8 + 8, 0:D],
                        ps[:].rearrange("p (h d) -> p h d", d=D),
                    )

            # ---------------- q/k chain helper ----------------
            chain_idx = [0]

            def qk_chain(jc, ms, force_gpsimd=False):
                # out = (w_jc)^T @ x^T -> [j-dims on partitions, tokens]
                ps = ps1.tile([128, 512], F32, tag="ps1")
                for ct in range(NCT):
                    nc.tensor.matmul(
                        ps[:],
                        w_sb[:, ct, jc * 128 : (jc + 1) * 128],
                        x_sb[:, ct, ms : ms + 512],
                        start=(ct == 0),
                        stop=(ct == NCT - 1),
                    )
                # RoPE in [d, n] layout: qh = q*cos + shuf(q)*sin'
                qs = qsp.tile([128, 512], F32, tag="qs")
                nc.vector.stream_shuffle(qs[:], ps[:], SHUF_MASK)
                a = rtp.tile([128, 512], BF16, tag="ra")
                nc.vector.tensor_tensor(
                    out=a[:], in0=ps[:], in1=cos_sb[:, ms : ms + 512], op=OP.mult
                )
                b = rtp.tile([128, 512], BF16, tag="rb")
                heavy = (not force_gpsimd) and chain_idx[0] % 2 == 0
                eng = nc.vector if heavy else nc.gpsimd
                eng.tensor_tensor(
                    out=b[:], in0=qs[:], in1=sin_sb[:, ms : ms + 512], op=OP.mult
                )
                eng.tensor_tensor(
                    out=qkT[:, jc, ms : ms + 512], in0=a[:], in1=b[:], op=OP.add
                )
                chain_idx[0] += 1

            def pair_chains(p, lo, hi, force_gpsimd=False):
                for jc, ms in [(p, 0), (p, 512), (8 + p, 0), (8 + p, 512)][lo:hi]:
                    qk_chain(jc, ms, force_gpsimd)

            # ---------------- attention block ----------------
            def attention(p, mc):
                ms = mc * 512
                pT = ptp.tile([128, ntile, 2, 512], BF16, tag="pT")
                for t in range(ntile):
                    g = grp.tile([128, 2, 512], F32, tag="g")
                    for par in range(2):
                        lo, hi = par * 64, par * 64 + 64
                        nc.tensor.matmul(
                            g[:, par, :],
                            qkT[lo:hi, 8 + p, t * 128 : (t + 1) * 128],
                            qkT[lo:hi, p, ms : ms + 512],
                            start=True,
                            stop=True,
                        )
                    nc.scalar.activation(
                        pT[:, t, :, :], g[:], AF.Exp, scale=SCALE
                    )
                if DEBUG and p == 0 and mc == 0:
                    nc.sync.dma_start(
                        dbg_pT.rearrange(
                            "q (t a m) -> q t a m", t=ntile, a=2
                        ),
                        pT[:],
                    )
                for par in range(2):
                    pot = pop.tile([65, 512], F32, tag="po")
                    for t in range(ntile):
                        nc.tensor.matmul(
                            pot[:],
                            v_sb[:, t, 2 * p + par, :],
                            pT[:, t, par, :],
                            start=(t == 0),
                            stop=(t == ntile - 1),
                        )
                    # free the PSUM bank fast: copy numerator + denominator
                    # out, then normalize SBUF-side off the PV critical path
                    # free the PSUM bank fast (num+hop copies), then the den
                    # path runs off the PV critical path
                    num = nrm.tile([64, 512], BF16, tag="num")
                    nc.vector.tensor_copy(num[:, :], pot[0:64, :])
                    hop = nrm.tile([65, 512], F32, tag="hop")
                    nc.vector.tensor_copy(hop[64:65, :], pot[64:65, :])
                    nc.sync.dma_start(hop[0:1, :], hop[64:65, :])
                    nc.vector.reciprocal_approx_fast(
                        out=hop[0:1, :], in_=hop[0:1, :]
                    )
                    den = nrm.tile([64, 512], F32, tag="den")
                    nc.gpsimd.partition_broadcast(den[:, :], hop[0:1, :])
                    if par == 0:
                        nc.vector.tensor_tensor(
                            out=oT[0:64, p, ms : ms + 512],
                            in0=num[:, :],
                            in1=den[:, :],
                            op=OP.mult,
                        )
                    else:
                        tmpo = nrm.tile([64, 512], BF16, tag="tmpo")
                        nc.vector.tensor_tensor(
                            out=tmpo[:],
                            in0=num[:, :],
                            in1=den[:, :],
                            op=OP.mult,
                        )
                        nc.sync.dma_start(
                            oT[64:128, p, ms : ms + 512], tmpo[:]
                        )

            # ---------------- pipelined schedule ----------------
            pair_chains(0, 0, 4)
            pair_chains(1, 0, 4)
            for p in range(8):
                attention(p, 0)
                if p + 2 < 8:
                    pair_chains(p + 2, 0, 2)
                attention(p, 1)
                if p + 2 < 8:
                    pair_chains(p + 2, 2, 4)
                if nmc > 2:
                    for mc in range(2, nmc):
                        attention(p, mc)

            if DEBUG:
                nc.sync.dma_start(
                    dbg_qkT.rearrange("p (a b) -> p a b", a=16), qkT[:]
                )
                nc.sync.dma_start(
                    dbg_v.rearrange(
                        "p (t h d) -> p t h d", t=n_tok // 128, h=H
                    ),
                    v_sb[:],
                )
                nc.sync.dma_start(
                    dbg_oT.rearrange("p (a b) -> p a b", a=NCT), oT[:]
                )

            # ---------------- proj ----------------
            for t in range(ntile):
                for ec in range(2):
                    ps = ps1.tile([128, 512], F32, tag="ps1")
                    for ft in range(NCT):
                        nc.tensor.matmul(
                            ps[:],
                            oT[:, ft, t * 128 : (t + 1) * 128],
                            pw_sb[:, ft, ec, :],
                            start=(ft == 0),
                            stop=(ft == NCT - 1),
                        )
                    ysb = yp.tile([128, 512], F32, tag="y")
                    nc.vector.tensor_tensor(
                        out=ysb[:],
                        in0=ps[:],
                        in1=bias_b[:, ec * 512 : (ec + 1) * 512],
                        op=OP.add,
                    )
                    nc.scalar.dma_start(
                        y[t * 128 : (t + 1) * 128, ec * 512 : (ec + 1) * 512],
                        ysb[:],
                    )

    nc.compile()
    return nc


def _host_inputs(x, rope_freqs, qkv_w, proj_w, proj_b):
    import ml_dtypes

    x = np.asarray(x, dtype=np.float32)
    rope_freqs = np.asarray(rope_freqs, dtype=np.float32)
    qkv_w = np.asarray(qkv_w, dtype=np.float32)
    proj_w = np.asarray(proj_w, dtype=np.float32)
    proj_b = np.asarray(proj_b, dtype=np.float32)

    B, n_tok, _ = x.shape
    perm = np.asarray(PERM64)

    wTh = np.ascontiguousarray(qkv_w.T)  # [C, 3C]
    # permute q,k head-dim columns so rope partners are 16 partitions apart
    for blk in range(2):
        j0 = blk * C
        wTh[:, j0 : j0 + C] = (
            wTh[:, j0 : j0 + C].reshape(C, H, D)[:, :, perm].reshape(C, C)
        )
    wTh = wTh.astype(ml_dtypes.bfloat16)
    pwTh = np.ascontiguousarray(proj_w.T).astype(ml_dtypes.bfloat16)

    def part_major(a):
        # [C, J] -> [128, (C//128)*J]: partition-major for fat DMA descriptors
        J = a.shape[1]
        return np.ascontiguousarray(
            a.reshape(C // 128, 128, J).transpose(1, 0, 2).reshape(128, -1)
        )

    wvh = part_major(wTh[:, 2 * C :])
    wqkh = part_major(wTh[:, : 2 * C])
    pwPh = part_major(pwTh)

    freqs = rope_freqs[0, :, 0, :]  # [N, D]
    dsel = perm[np.arange(128) % 64]
    sign = np.where((np.arange(128) % 32) < 16, -1.0, 1.0).astype(np.float32)
    cosTh = np.ascontiguousarray(np.cos(freqs[:, dsel]).T).astype(
        ml_dtypes.bfloat16
    )  # [128, N]
    sinTh = np.ascontiguousarray(
        np.sin(freqs[:, dsel]).T * sign[:, None]
    ).astype(ml_dtypes.bfloat16)
    pbh = np.ascontiguousarray(proj_b.reshape(1, C)).astype(ml_dtypes.bfloat16)

    in_maps = []
    for b in range(B):
        xb = np.ascontiguousarray(x[b].T).astype(ml_dtypes.bfloat16)
        in_maps.append(
            {
                "xP": part_major(xb),
                "wv": wvh,
                "wqk": wqkh,
                "pwP": pwPh,
                "pbias": pbh,
                "cosT": cosTh,
                "sinT": sinTh,
            }
        )
    return in_maps, n_tok


def kernel(x, rope_freqs, qkv_w, proj_w, proj_b):
    global LAST_EXEC_NS
    in_maps, n_tok = _host_inputs(x, rope_freqs, qkv_w, proj_w, proj_b)
    key = ("nc", n_tok)
    if key not in _CACHE:
        _CACHE[key] = build(n_tok)
    nc = _CACHE[key]

    trace = False
    if PROFILE:
        try:
            import profshim

            profshim.install()
            trace = True
        except Exception:
            trace = False

    res = run_bass_kernel_spmd(
        nc, in_maps, list(range(len(in_maps))), trace=trace, tmpdir=TRACE_DIR
    )
    LAST_EXEC_NS = res.exec_time_ns
    out = np.stack([res.results[b]["y"] for b in range(len(in_maps))], axis=0)
    return out


# revision 46
# speedup vs baseline: 1.0031x; 1.0031x over previous
"""Self-contained Trainium2 Bass kernel for nn_Attention_395136991961.

Dense multi-head attention (B=8, N=1024, C=1024, H=16, D=64) with RoPE,
full materialized softmax, and output projection.

Sharding: data-parallel over batch B across the 8 NeuronCores (one batch
element per core, weights replicated, no collectives).

v2 design (vs the f32r baseline at ~467us):
  - everything bf16 into the PE (measured numerics: ~4e-3 relmax, gate 2e-2)
  - q/k produced TRANSPOSED directly by the QKV matmul (w stationary,
    x^T moving) -> zero PE transposes, no PE-waits-DVE serialization
  - RoPE applied in the [d-on-partitions, n] layout: the rotate-half
    partner lives 16 partitions away inside a 32-group thanks to a host-
    side permutation of the q/k weight columns (contraction order of
    q.k is permutation invariant), so one DVE stream_shuffle + two
    multiplies + one add do RoPE at full partition utilization
  - softmax exp in [128, 2x512] ops (2 PSUM banks) straight PSUM->SBUF
  - attention blocks software-pipelined with the QKV chains of later
    head-pairs so the exp-gated PE gaps are filled with matmul work
  - PSUM budget: 2 banks chains (qkv+proj shared), 4 banks logit groups,
    2 banks PV accumulators = 8
"""

import sys

if "/opt/trn_rl_repo" not in sys.path:
    sys.path.insert(0, "/opt/trn_rl_repo")

import numpy as np

import concourse.tile as tile
import concourse.mybir as mybir
from concourse import bacc
from concourse.bass_utils import run_bass_kernel_spmd

F32 = mybir.dt.float32
BF16 = mybir.dt.bfloat16
AF = mybir.ActivationFunctionType
OP = mybir.AluOpType

N_CORES = 8
C = 1024
H = 16
D = 64
NCT = C // 128          # contraction chunks (8)
SCALE = float(D) ** -0.5

# rotate-half partner permutation: partition p (within a 64-half) holds
# d = PERM64[p]; partner (d <-> d+-32) sits at p XOR 16 (same 32-group)
PERM64 = (
    list(range(0, 16)) + list(range(32, 48))
    + list(range(16, 32)) + list(range(48, 64))
)
SHUF_MASK = [i ^ 16 for i in range(32)]

PROFILE = False
TRACE_DIR = None
DEBUG = False
LAST_EXEC_NS = None
_CACHE = {}


def build(n_tok):
    ntile = n_tok // 128
    nmc = n_tok // 512           # token chunks for logits moving dim

    nc = bacc.Bacc("TRN2", target_bir_lowering=False, debug=False, num_devices=1)

    # partition-major layouts (host pre-permuted) for large-descriptor DMAs
    xP = nc.dram_tensor("xP", [128, NCT * n_tok], BF16, kind="ExternalInput").ap()
    wv = nc.dram_tensor("wv", [128, NCT * C], BF16, kind="ExternalInput").ap()
    wqk = nc.dram_tensor(
        "wqk", [128, NCT * 2 * C], BF16, kind="ExternalInput"
    ).ap()
    pwP = nc.dram_tensor("pwP", [128, NCT * C], BF16, kind="ExternalInput").ap()
    pbias = nc.dram_tensor("pbias", [1, C], BF16, kind="ExternalInput").ap()
    cosT = nc.dram_tensor("cosT", [128, n_tok], BF16, kind="ExternalInput").ap()
    sinT = nc.dram_tensor("sinT", [128, n_tok], BF16, kind="ExternalInput").ap()
    y = nc.dram_tensor("y", [n_tok, C], F32, kind="ExternalOutput").ap()
    if DEBUG:
        dbg_qkT = nc.dram_tensor(
            "dbg_qkT", [128, 16 * n_tok], BF16, kind="ExternalOutput"
        ).ap()
        dbg_v = nc.dram_tensor(
            "dbg_v", [128, (n_tok // 128) * H * (D + 1)], BF16,
            kind="ExternalOutput",
        ).ap()
        dbg_oT = nc.dram_tensor(
            "dbg_oT", [128, NCT * n_tok], BF16, kind="ExternalOutput"
        ).ap()
        dbg_pT = nc.dram_tensor(
            "dbg_pT", [128, (n_tok // 128) * 2 * 512], BF16,
            kind="ExternalOutput",
        ).ap()

    xP_t = xP.rearrange("p (t n) -> p t n", t=NCT)
    wv_t = wv.rearrange("p (t j) -> p t j", t=NCT)
    wqk_t = wqk.rearrange("p (t j) -> p t j", t=NCT)
    pwP_t = pwP.rearrange("p (t a e) -> p t a e", t=NCT, a=2)

    with tile.TileContext(nc) as tc:
        with (
            tc.tile_pool(name="persist", bufs=1) as pp,
            tc.tile_pool(name="ptp", bufs=2) as ptp,
            tc.tile_pool(name="qsp", bufs=2) as qsp,
            tc.tile_pool(name="rtp", bufs=2) as rtp,
            tc.tile_pool(name="nrm", bufs=2) as nrm,
            tc.tile_pool(name="ypool", bufs=2) as yp,
            tc.tile_pool(name="ps1", bufs=2, space="PSUM") as ps1,
            tc.tile_pool(name="grp", bufs=2, space="PSUM") as grp,
            tc.tile_pool(name="pop", bufs=2, space="PSUM") as pop,
        ):
            # ---------------- persistent tiles + loads ----------------
            x_sb = pp.tile([128, NCT, n_tok], BF16, tag="x")
            w_sb = pp.tile([128, NCT, 3 * C], BF16, tag="w")
            qkT = pp.tile([128, 16, n_tok], BF16, tag="qkT")
            v_sb = pp.tile([128, ntile, H, D + 1], BF16, tag="v")
            oT = pp.tile([128, NCT, n_tok], BF16, tag="oT")
            pw_sb = pp.tile([128, NCT, 2, 512], BF16, tag="pw")
            cos_sb = pp.tile([128, n_tok], BF16, tag="cos")
            sin_sb = pp.tile([128, n_tok], BF16, tag="sin")
            bias_b = pp.tile([128, C], BF16, tag="biasb")

            nc.scalar.dma_start(cos_sb[:], cosT[:])
            nc.scalar.dma_start(sin_sb[:], sinT[:])
            nc.scalar.dma_start(bias_b[0:1, :], pbias[:])
            nc.gpsimd.partition_broadcast(bias_b[:, :], bias_b[0:1, :])
            nc.scalar.dma_start(x_sb[:], xP_t)
            # v columns first, then q/k in consumption order
            nc.sync.dma_start(w_sb[:, :, 2 * C : 3 * C], wv_t)
            for j0 in range(0, 2 * C, 512):
                nc.sync.dma_start(
                    w_sb[:, :, j0 : j0 + 512], wqk_t[:, :, j0 : j0 + 512]
                )
            nc.scalar.dma_start(pw_sb[:], pwP_t)
            nc.vector.memset(v_sb[:, :, :, D : D + 1], 1.0)

            # ---------------- v chains (stationary x, moving w) ----------
            for t in range(ntile):
                for half in range(2):
                    ps = ps1.tile([128, 512], F32, tag="ps1")
                    j0 = 2 * C + half * 512
                    for ct in range(NCT):
                        nc.tensor.matmul(
                            ps[:],
                            x_sb[:, ct, t * 128 : (t + 1) * 128],
                            w_sb[:, ct, j0 : j0 + 512],
                            start=(ct == 0),
                            stop=(ct == NCT - 1),
                        )
                    nc.scalar.copy(
                        v_sb[:, t, half * 8 : half * 8 + 8, 0:D],
                        ps[:].rearrange("p (h d) -> p h d", d=D),
                    )

            # ---------------- q/k chain helper ----------------
            chain_idx = [0]

            def qk_chain(jc, ms, force_gpsimd=False):
                # out = (w_jc)^T @ x^T -> [j-dims on partitions, tokens]
                ps = ps1.tile([128, 512], F32, tag="ps1")
                for ct in range(NCT):
                    nc.tensor.matmul(
                        ps[:],
                        w_sb[:, ct, jc * 128 : (jc + 1) * 128],
                        x_sb[:, ct, ms : ms + 512],
                        start=(ct == 0),
                        stop=(ct == NCT - 1),
                    )
                # RoPE in [d, n] layout: qh = q*cos + shuf(q)*sin'
                qs = qsp.tile([128, 512], F32, tag="qs")
                nc.vector.stream_shuffle(qs[:], ps[:], SHUF_MASK)
                a = rtp.tile([128, 512], BF16, tag="ra")
                nc.vector.tensor_tensor(
                    out=a[:], in0=ps[:], in1=cos_sb[:, ms : ms + 512], op=OP.mult
                )
                b = rtp.tile([128, 512], BF16, tag="rb")
                heavy = (not force_gpsimd) and chain_idx[0] % 2 == 0
                eng = nc.vector if heavy else nc.gpsimd
                eng.tensor_tensor(
                    out=b[:], in0=qs[:], in1=sin_sb[:, ms : ms + 512], op=OP.mult
                )
                eng.tensor_tensor(
                    out=qkT[:, jc, ms : ms + 512], in0=a[:], in1=b[:], op=OP.add
                )
                chain_idx[0] += 1

            def pair_chains(p, lo, hi, force_gpsimd=False):
                for jc, ms in [(p, 0), (p, 512), (8 + p, 0), (8 + p, 512)][lo:hi]:
                    qk_chain(jc, ms, force_gpsimd)

            # ---------------- attention block ----------------
            def attention(p, mc):
                ms = mc * 512
                pT = ptp.tile([128, ntile, 2, 512], BF16, tag="pT")
                for t in range(ntile):
                    g = grp.tile([128, 2, 512], F32, tag="g")
                    for par in range(2):
                        lo, hi = par * 64, par * 64 + 64
                        nc.tensor.matmul(
                            g[:, par, :],
                            qkT[lo:hi, 8 + p, t * 128 : (t + 1) * 128],
                            qkT[lo:hi, p, ms : ms + 512],
                            start=True,
                            stop=True,
                        )
                    nc.scalar.activation(
                        pT[:, t, :, :], g[:], AF.Exp, scale=SCALE
                    )
                if DEBUG and p == 0 and mc == 0:
                    nc.sync.dma_start(
                        dbg_pT.rearrange(
                            "q (t a m) -> q t a m", t=ntile, a=2
                        ),
                        pT[:],
                    )
                for par in range(2):
                    pot = pop.tile([65, 512], F32, tag="po")
                    for t in range(ntile):
                        nc.tensor.matmul(
                            pot[:],
                            v_sb[:, t, 2 * p + par, :],
                            pT[:, t, par, :],
                            start=(t == 0),
                            stop=(t == ntile - 1),
                        )
                    # free the PSUM bank fast: copy numerator + denominator
                    # out, then normalize SBUF-side off the PV critical path
                    # free the PSUM bank fast (num+hop copies), then the den
                    # path runs off the PV critical path
                    num = nrm.tile([64, 512], BF16, tag="num")
                    nc.vector.tensor_copy(num[:, :], pot[0:64, :])
                    hop = nrm.tile([65, 512], F32, tag="hop")
                    nc.vector.tensor_copy(hop[64:65, :], pot[64:65, :])
                    nc.sync.dma_start(hop[0:1, :], hop[64:65, :])
                    nc.vector.reciprocal_approx_fast(
                        out=hop[0:1, :], in_=hop[0:1, :]
                    )
                    den = nrm.tile([64, 512], F32, tag="den")
                    nc.gpsimd.partition_broadcast(den[:, :], hop[0:1, :])
                    if par == 0:
                        nc.vector.tensor_tensor(
                            out=oT[0:64, p, ms : ms + 512],
                            in0=num[:, :],
                            in1=den[:, :],
                            op=OP.mult,
                        )
                    else:
                        tmpo = nrm.tile([64, 512], BF16, tag="tmpo")
                        nc.vector.tensor_tensor(
                            out=tmpo[:],
                            in0=num[:, :],
                            in1=den[:, :],
                            op=OP.mult,
                        )
                        nc.sync.dma_start(
                            oT[64:128, p, ms : ms + 512], tmpo[:]
                        )

            # ---------------- pipelined schedule ----------------
            pair_chains(0, 0, 4)
            pair_chains(1, 0, 4)
            for p in range(8):
                attention(p, 0)
                if p + 2 < 8:
                    pair_chains(p + 2, 0, 2)
                attention(p, 1)
                if p + 2 < 8:
                    pair_chains(p + 2, 2, 4)
                if nmc > 2:
                    for mc in range(2, nmc):
                        attention(p, mc)

            if DEBUG:
                nc.sync.dma_start(
                    dbg_qkT.rearrange("p (a b) -> p a b", a=16), qkT[:]
                )
                nc.sync.dma_start(
                    dbg_v.rearrange(
                        "p (t h d) -> p t h d", t=n_tok // 128, h=H
                    ),
                    v_sb[:],
                )
                nc.sync.dma_start(
                    dbg_oT.rearrange("p (a b) -> p a b", a=NCT), oT[:]
                )

            # ---------------- proj ----------------
            for t in range(ntile):
                for ec in range(2):
                    ps = ps1.tile([128, 512], F32, tag="ps1")
                    for ft in range(NCT):
                        nc.tensor.matmul(
                            ps[:],
                            oT[:, ft, t * 128 : (t + 1) * 128],
                            pw_sb[:, ft, ec, :],
                            start=(ft == 0),
                            stop=(ft == NCT - 1),
                        )
                    ysb = yp.tile([128, 512], F32, tag="y")
                    nc.vector.tensor_tensor(
                        out=ysb[:],
                        in0=ps[:],
                        in1=bias_b[:, ec * 512 : (ec + 1) * 512],
                        op=OP.add,
                    )
                    nc.scalar.dma_start(
                        y[t * 128 : (t + 1) * 128, ec * 512 : (ec + 1) * 512],
                        ysb[:],
                    )

    nc.compile()
    return nc


def _host_inputs(x, rope_freqs, qkv_w, proj_w, proj_b):
    import ml_dtypes

    x = np.asarray(x, dtype=np.float32)
    rope_freqs = np.asarray(rope_freqs, dtype=np.float32)
    qkv_w = np.asarray(qkv_w, dtype=np.float32)
    proj_w = np.asarray(proj_w, dtype=np.float32)
    proj_b = np.asarray(proj_b, dtype=np.float32)

    B, n_tok, _ = x.shape
    perm = np.asarray(PERM64)

    wTh = np.ascontiguousarray(qkv_w.T)  # [C, 3C]
    # permute q,k head-dim columns so rope partners are 16 partitions apart
    for blk in range(2):
        j0 = blk * C
        wTh[:, j0 : j0 + C] = (
            wTh[:, j0 : j0 + C].reshape(C, H, D)[:, :, perm].reshape(C, C)
        )
    wTh = wTh.astype(ml_dtypes.bfloat16)
    pwTh = np.ascontiguousarray(proj_w.T).astype(ml_dtypes.bfloat16)

    def part_major(a):
        # [C, J] -> [128, (C//128)*J]: partition-major for fat DMA descriptors
        J = a.shape[1]
        return np.ascontiguousarray(
            a.reshape(C // 128, 128, J).transpose(1, 0, 2).reshape(128, -1)
        )

    wvh = part_major(wTh[:, 2 * C :])
    wqkh = part_major(wTh[:, : 2 * C])
    pwPh = part_major(pwTh)

    freqs = rope_freqs[0, :, 0, :]  # [N, D]
    dsel = perm[np.arange(128) % 64]
    sign = np.where((np.arange(128) % 32) < 16, -1.0, 1.0).astype(np.float32)
    cosTh = np.ascontiguousarray(np.cos(freqs[:, dsel]).T).astype(
        ml_dtypes.bfloat16
    )  # [128, N]
    sinTh = np.ascontiguousarray(
        np.sin(freqs[:, dsel]).T * sign[:, None]
    ).astype(ml_dtypes.bfloat16)
    pbh = np.ascontiguousarray(proj_b.reshape(1, C)).astype(ml_dtypes.bfloat16)

    in_maps = []
    for b in range(B):
        xb = np.ascontiguousarray(x[b].T).astype(ml_dtypes.bfloat16)
        in_maps.append(
            {
                "xP": part_major(xb),
                "wv": wvh,
                "wqk": wqkh,
                "pwP": pwPh,
                "pbias": pbh,
                "cosT": cosTh,
                "sinT": sinTh,
            }
        )
    return in_maps, n_tok


def kernel(x, rope_freqs, qkv_w, proj_w, proj_b):
    global LAST_EXEC_NS
    in_maps, n_tok = _host_inputs(x, rope_freqs, qkv_w, proj_w, proj_b)
    key = ("nc", n_tok)
    if key not in _CACHE:
        _CACHE[key] = build(n_tok)
    nc = _CACHE[key]

    trace = False
    if PROFILE:
        try:
            import profshim

            profshim.install()
            trace = True
        except Exception:
            trace = False

    res = run_bass_kernel_spmd(
        nc, in_maps, list(range(len(in_maps))), trace=trace, tmpdir=TRACE_DIR
    )
    LAST_EXEC_NS = res.exec_time_ns
    out = np.stack([res.results[b]["y"] for b in range(len(in_maps))], axis=0)
    return out


# revision 49
# speedup vs baseline: 1.0156x; 1.0125x over previous
"""Self-contained Trainium2 Bass kernel for nn_Attention_395136991961.

Dense multi-head attention (B=8, N=1024, C=1024, H=16, D=64) with RoPE,
full materialized softmax, and output projection.

Sharding: data-parallel over batch B across the 8 NeuronCores (one batch
element per core, weights replicated, no collectives).

v2 design (vs the f32r baseline at ~467us):
  - everything bf16 into the PE (measured numerics: ~4e-3 relmax, gate 2e-2)
  - q/k produced TRANSPOSED directly by the QKV matmul (w stationary,
    x^T moving) -> zero PE transposes, no PE-waits-DVE serialization
  - RoPE applied in the [d-on-partitions, n] layout: the rotate-half
    partner lives 16 partitions away inside a 32-group thanks to a host-
    side permutation of the q/k weight columns (contraction order of
    q.k is permutation invariant), so one DVE stream_shuffle + two
    multiplies + one add do RoPE at full partition utilization
  - softmax exp in [128, 2x512] ops (2 PSUM banks) straight PSUM->SBUF
  - attention blocks software-pipelined with the QKV chains of later
    head-pairs so the exp-gated PE gaps are filled with matmul work
  - PSUM budget: 2 banks chains (qkv+proj shared), 4 banks logit groups,
    2 banks PV accumulators = 8
"""

import sys

if "/opt/trn_rl_repo" not in sys.path:
    sys.path.insert(0, "/opt/trn_rl_repo")

import numpy as np

import concourse.tile as tile
import concourse.mybir as mybir
from concourse import bacc
from concourse.bass_utils import run_bass_kernel_spmd

F32 = mybir.dt.float32
BF16 = mybir.dt.bfloat16
AF = mybir.ActivationFunctionType
OP = mybir.AluOpType

N_CORES = 8
C = 1024
H = 16
D = 64
NCT = C // 128          # contraction chunks (8)
SCALE = float(D) ** -0.5

# rotate-half partner permutation: partition p (within a 64-half) holds
# d = PERM64[p]; partner (d <-> d+-32) sits at p XOR 16 (same 32-group)
PERM64 = (
    list(range(0, 16)) + list(range(32, 48))
    + list(range(16, 32)) + list(range(48, 64))
)
SHUF_MASK = [i ^ 16 for i in range(32)]

PROFILE = False
TRACE_DIR = None
DEBUG = False
LAST_EXEC_NS = None
_CACHE = {}


def build(n_tok):
    ntile = n_tok // 128
    nmc = n_tok // 512           # token chunks for logits moving dim

    nc = bacc.Bacc("TRN2", target_bir_lowering=False, debug=False, num_devices=1)

    # partition-major layouts (host pre-permuted) for large-descriptor DMAs
    xP = nc.dram_tensor("xP", [128, NCT * n_tok], BF16, kind="ExternalInput").ap()
    wv = nc.dram_tensor("wv", [128, NCT * C], BF16, kind="ExternalInput").ap()
    wqk = nc.dram_tensor(
        "wqk", [128, NCT * 2 * C], BF16, kind="ExternalInput"
    ).ap()
    pwP = nc.dram_tensor("pwP", [128, NCT * C], BF16, kind="ExternalInput").ap()
    pbias = nc.dram_tensor("pbias", [1, C], BF16, kind="ExternalInput").ap()
    cosT = nc.dram_tensor("cosT", [128, n_tok], BF16, kind="ExternalInput").ap()
    sinT = nc.dram_tensor("sinT", [128, n_tok], BF16, kind="ExternalInput").ap()
    y = nc.dram_tensor("y", [n_tok, C], F32, kind="ExternalOutput").ap()
    if DEBUG:
        dbg_qkT = nc.dram_tensor(
            "dbg_qkT", [128, 16 * n_tok], BF16, kind="ExternalOutput"
        ).ap()
        dbg_v = nc.dram_tensor(
            "dbg_v", [128, (n_tok // 128) * H * (D + 1)], BF16,
            kind="ExternalOutput",
        ).ap()
        dbg_oT = nc.dram_tensor(
            "dbg_oT", [128, NCT * n_tok], BF16, kind="ExternalOutput"
        ).ap()
        dbg_pT = nc.dram_tensor(
            "dbg_pT", [128, (n_tok // 128) * 2 * 512], BF16,
            kind="ExternalOutput",
        ).ap()

    xP_t = xP.rearrange("p (t n) -> p t n", t=NCT)
    wv_t = wv.rearrange("p (t j) -> p t j", t=NCT)
    wqk_t = wqk.rearrange("p (t j) -> p t j", t=NCT)
    pwP_t = pwP.rearrange("p (t a e) -> p t a e", t=NCT, a=2)

    with tile.TileContext(nc) as tc:
        with (
            tc.tile_pool(name="persist", bufs=1) as pp,
            tc.tile_pool(name="ptp", bufs=2) as ptp,
            tc.tile_pool(name="qsp", bufs=2) as qsp,
            tc.tile_pool(name="rtp", bufs=2) as rtp,
            tc.tile_pool(name="nrm", bufs=2) as nrm,
            tc.tile_pool(name="denp", bufs=4) as denp,
            tc.tile_pool(name="ypool", bufs=2) as yp,
            tc.tile_pool(name="ps1", bufs=2, space="PSUM") as ps1,
            tc.tile_pool(name="grp", bufs=2, space="PSUM") as grp,
            tc.tile_pool(name="pop", bufs=2, space="PSUM") as pop,
        ):
            # ---------------- persistent tiles + loads ----------------
            x_sb = pp.tile([128, NCT, n_tok], BF16, tag="x")
            w_sb = pp.tile([128, NCT, 3 * C], BF16, tag="w")
            qkT = pp.tile([128, 16, n_tok], BF16, tag="qkT")
            v_sb = pp.tile([128, ntile, H, D + 1], BF16, tag="v")
            oT = pp.tile([128, NCT, n_tok], BF16, tag="oT")
            pw_sb = pp.tile([128, NCT, 2, 512], BF16, tag="pw")
            cos_sb = pp.tile([128, n_tok], BF16, tag="cos")
            sin_sb = pp.tile([128, n_tok], BF16, tag="sin")
            bias_b = pp.tile([128, C], BF16, tag="biasb")

            nc.scalar.dma_start(cos_sb[:], cosT[:])
            nc.scalar.dma_start(sin_sb[:], sinT[:])
            nc.scalar.dma_start(bias_b[0:1, :], pbias[:])
            nc.gpsimd.partition_broadcast(bias_b[:, :], bias_b[0:1, :])
            nc.scalar.dma_start(x_sb[:], xP_t)
            # v columns first, then q/k in consumption order
            nc.sync.dma_start(w_sb[:, :, 2 * C : 3 * C], wv_t)
            for j0 in range(0, 2 * C, 512):
                nc.sync.dma_start(
                    w_sb[:, :, j0 : j0 + 512], wqk_t[:, :, j0 : j0 + 512]
                )
            nc.scalar.dma_start(pw_sb[:], pwP_t)
            nc.vector.memset(v_sb[:, :, :, D : D + 1], 1.0)

            # ---------------- v chains (stationary x, moving w) ----------
            for t in range(ntile):
                for half in range(2):
                    ps = ps1.tile([128, 512], F32, tag="ps1")
                    j0 = 2 * C + half * 512
                    for ct in range(NCT):
                        nc.tensor.matmul(
                            ps[:],
                            x_sb[:, ct, t * 128 : (t + 1) * 128],
                            w_sb[:, ct, j0 : j0 + 512],
                            start=(ct == 0),
                            stop=(ct == NCT - 1),
                        )
                    nc.scalar.copy(
                        v_sb[:, t, half * 8 : half * 8 + 8, 0:D],
                        ps[:].rearrange("p (h d) -> p h d", d=D),
                    )

            # ---------------- q/k chain helper ----------------
            chain_idx = [0]

            def qk_chain(jc, ms, force_gpsimd=False):
                # out = (w_jc)^T @ x^T -> [j-dims on partitions, tokens]
                ps = ps1.tile([128, 512], F32, tag="ps1")
                for ct in range(NCT):
                    nc.tensor.matmul(
                        ps[:],
                        w_sb[:, ct, jc * 128 : (jc + 1) * 128],
                        x_sb[:, ct, ms : ms + 512],
                        start=(ct == 0),
                        stop=(ct == NCT - 1),
                    )
                # RoPE in [d, n] layout: qh = q*cos + shuf(q)*sin'
                qs = qsp.tile([128, 512], F32, tag="qs")
                nc.vector.stream_shuffle(qs[:], ps[:], SHUF_MASK)
                a = rtp.tile([128, 512], BF16, tag="ra")
                nc.vector.tensor_tensor(
                    out=a[:], in0=ps[:], in1=cos_sb[:, ms : ms + 512], op=OP.mult
                )
                b = rtp.tile([128, 512], BF16, tag="rb")
                heavy = (not force_gpsimd) and chain_idx[0] % 2 == 0
                eng = nc.vector if heavy else nc.gpsimd
                eng.tensor_tensor(
                    out=b[:], in0=qs[:], in1=sin_sb[:, ms : ms + 512], op=OP.mult
                )
                eng.tensor_tensor(
                    out=qkT[:, jc, ms : ms + 512], in0=a[:], in1=b[:], op=OP.add
                )
                chain_idx[0] += 1

            def pair_chains(p, lo, hi, force_gpsimd=False):
                for jc, ms in [(p, 0), (p, 512), (8 + p, 0), (8 + p, 512)][lo:hi]:
                    qk_chain(jc, ms, force_gpsimd)

            # ---------------- attention block ----------------
            def attention(p, mc):
                ms = mc * 512
                pT = ptp.tile([128, ntile, 2, 512], BF16, tag="pT")
                for t in range(ntile):
                    g = grp.tile([128, 2, 512], F32, tag="g")
                    for par in range(2):
                        lo, hi = par * 64, par * 64 + 64
                        nc.tensor.matmul(
                            g[:, par, :],
                            qkT[lo:hi, 8 + p, t * 128 : (t + 1) * 128],
                            qkT[lo:hi, p, ms : ms + 512],
                            start=True,
                            stop=True,
                        )
                    nc.scalar.activation(
                        pT[:, t, :, :], g[:], AF.Exp, scale=SCALE
                    )
                if DEBUG and p == 0 and mc == 0:
                    nc.sync.dma_start(
                        dbg_pT.rearrange(
                            "q (t a m) -> q t a m", t=ntile, a=2
                        ),
                        pT[:],
                    )
                cur = []
                for par in range(2):
                    pot = pop.tile([65, 512], F32, tag="po")
                    for t in range(ntile):
                        nc.tensor.matmul(
                            pot[:],
                            v_sb[:, t, 2 * p + par, :],
                            pT[:, t, par, :],
                            start=(t == 0),
                            stop=(t == ntile - 1),
                        )
                    # free the PSUM bank fast: unnormalized numerator straight
                    # into oT + denominator row to SBUF; the 1/den multiply is
                    # lagged one block so it never waits in-queue
                    hop = nrm.tile([65, 512], F32, tag="hop")
                    nc.vector.tensor_copy(hop[64:65, :], pot[64:65, :])
                    if par == 0:
                        nc.vector.tensor_copy(
                            oT[0:64, p, ms : ms + 512], pot[0:64, :]
                        )
                    else:
                        tmpo = nrm.tile([64, 512], BF16, tag="tmpo")
                        nc.vector.tensor_copy(tmpo[:, :], pot[0:64, :])
                        nc.sync.dma_start(
                            oT[64:128, p, ms : ms + 512], tmpo[:]
                        )
                    nc.sync.dma_start(hop[0:1, :], hop[64:65, :])
                    nc.vector.reciprocal_approx_fast(
                        out=hop[0:1, :], in_=hop[0:1, :]
                    )
                    denb = denp.tile([128, 512], BF16, tag="denb")
                    nc.vector.tensor_copy(denb[0:1, :], hop[0:1, :])
                    nc.gpsimd.partition_broadcast(denb[:, :], denb[0:1, :])
                    cur.append((par, p, ms, denb))
                # issue the PREVIOUS block's normalization multiplies (their
                # denominators are ready by now -> no engine-queue stall)
                for par_, p_, ms_, denb_ in pending:
                    lo = par_ * 64
                    nc.vector.tensor_tensor(
                        out=oT[lo : lo + 64, p_, ms_ : ms_ + 512],
                        in0=oT[lo : lo + 64, p_, ms_ : ms_ + 512],
                        in1=denb_[lo : lo + 64, :],
                        op=OP.mult,
                    )
                pending[:] = cur

            # ---------------- pipelined schedule ----------------
            pending = []
            pair_chains(0, 0, 4)
            pair_chains(1, 0, 4)
            for p in range(8):
                attention(p, 0)
                if p + 2 < 8:
                    pair_chains(p + 2, 0, 2)
                attention(p, 1)
                if p + 2 < 8:
                    pair_chains(p + 2, 2, 4)
                if nmc > 2:
                    for mc in range(2, nmc):
                        attention(p, mc)

            if DEBUG:
                nc.sync.dma_start(
                    dbg_qkT.rearrange("p (a b) -> p a b", a=16), qkT[:]
                )
                nc.sync.dma_start(
                    dbg_v.rearrange(
                        "p (t h d) -> p t h d", t=n_tok // 128, h=H
                    ),
                    v_sb[:],
                )
                nc.sync.dma_start(
                    dbg_oT.rearrange("p (a b) -> p a b", a=NCT), oT[:]
                )

            # flush the last block's normalization multiplies
            for par_, p_, ms_, denb_ in pending:
                lo = par_ * 64
                nc.vector.tensor_tensor(
                    out=oT[lo : lo + 64, p_, ms_ : ms_ + 512],
                    in0=oT[lo : lo + 64, p_, ms_ : ms_ + 512],
                    in1=denb_[lo : lo + 64, :],
                    op=OP.mult,
                )
            pending[:] = []

            # ---------------- proj ----------------
            for t in range(ntile):
                for ec in range(2):
                    ps = ps1.tile([128, 512], F32, tag="ps1")
                    for ft in range(NCT):
                        nc.tensor.matmul(
                            ps[:],
                            oT[:, ft, t * 128 : (t + 1) * 128],
                            pw_sb[:, ft, ec, :],
                            start=(ft == 0),
                            stop=(ft == NCT - 1),
                        )
                    ysb = yp.tile([128, 512], F32, tag="y")
                    nc.vector.tensor_tensor(
                        out=ysb[:],
                        in0=ps[:],
                        in1=bias_b[:, ec * 512 : (ec + 1) * 512],
                        op=OP.add,
                    )
                    nc.scalar.dma_start(
                        y[t * 128 : (t + 1) * 128, ec * 512 : (ec + 1) * 512],
                        ysb[:],
                    )

    nc.compile()
    return nc


def _host_inputs(x, rope_freqs, qkv_w, proj_w, proj_b):
    import ml_dtypes

    x = np.asarray(x, dtype=np.float32)
    rope_freqs = np.asarray(rope_freqs, dtype=np.float32)
    qkv_w = np.asarray(qkv_w, dtype=np.float32)
    proj_w = np.asarray(proj_w, dtype=np.float32)
    proj_b = np.asarray(proj_b, dtype=np.float32)

    B, n_tok, _ = x.shape
    perm = np.asarray(PERM64)

    wTh = np.ascontiguousarray(qkv_w.T)  # [C, 3C]
    # permute q,k head-dim columns so rope partners are 16 partitions apart
    for blk in range(2):
        j0 = blk * C
        wTh[:, j0 : j0 + C] = (
            wTh[:, j0 : j0 + C].reshape(C, H, D)[:, :, perm].reshape(C, C)
        )
    wTh = wTh.astype(ml_dtypes.bfloat16)
    pwTh = np.ascontiguousarray(proj_w.T).astype(ml_dtypes.bfloat16)

    def part_major(a):
        # [C, J] -> [128, (C//128)*J]: partition-major for fat DMA descriptors
        J = a.shape[1]
        return np.ascontiguousarray(
            a.reshape(C // 128, 128, J).transpose(1, 0, 2).reshape(128, -1)
        )

    wvh = part_major(wTh[:, 2 * C :])
    wqkh = part_major(wTh[:, : 2 * C])
    pwPh = part_major(pwTh)

    freqs = rope_freqs[0, :, 0, :]  # [N, D]
    dsel = perm[np.arange(128) % 64]
    sign = np.where((np.arange(128) % 32) < 16, -1.0, 1.0).astype(np.float32)
    cosTh = np.ascontiguousarray(np.cos(freqs[:, dsel]).T).astype(
        ml_dtypes.bfloat16
    )  # [128, N]
    sinTh = np.ascontiguousarray(
        np.sin(freqs[:, dsel]).T * sign[:, None]
    ).astype(ml_dtypes.bfloat16)
    pbh = np.ascontiguousarray(proj_b.reshape(1, C)).astype(ml_dtypes.bfloat16)

    in_maps = []
    for b in range(B):
        xb = np.ascontiguousarray(x[b].T).astype(ml_dtypes.bfloat16)
        in_maps.append(
            {
                "xP": part_major(xb),
                "wv": wvh,
                "wqk": wqkh,
                "pwP": pwPh,
                "pbias": pbh,
                "cosT": cosTh,
                "sinT": sinTh,
            }
        )
    return in_maps, n_tok


def kernel(x, rope_freqs, qkv_w, proj_w, proj_b):
    global LAST_EXEC_NS
    in_maps, n_tok = _host_inputs(x, rope_freqs, qkv_w, proj_w, proj_b)
    key = ("nc", n_tok)
    if key not in _CACHE:
        _CACHE[key] = build(n_tok)
    nc = _CACHE[key]

    trace = False
    if PROFILE:
        try:
            import profshim

            profshim.install()
            trace = True
        except Exception:
            trace = False

    res = run_bass_kernel_spmd(
        nc, in_maps, list(range(len(in_maps))), trace=trace, tmpdir=TRACE_DIR
    )
    LAST_EXEC_NS = res.exec_time_ns
    out = np.stack([res.results[b]["y"] for b in range(len(in_maps))], axis=0)
    return out
